# revision 1
# baseline (speedup 1.0000x reference)
"""TRN2 Bass kernel for nn_NaiveReweightedLoss (reweighted per-class BCE-style loss).

Reference semantics (N=32768 samples, C=1000 classes, t in {0,1}):
    B_c = sum_i t * softplus(-p),  C_c = sum_i (1-t) * softplus(p)
    n_pos_c = sum_i t, n_neg_c = N - n_pos_c
    valid = (n_pos>0)&(n_neg>0)
    loss = mean over valid classes of B/max(n_pos,1) + C/max(n_neg,1)

Device algorithm (data-parallel over rows, 8 cores x 4096 rows):
    Host re-encodes (byte-level only, no arithmetic):
      z8 = fp8e4m3(p) with sign bit XOR'd by t  (== fp8(c*p), c = 1-2t, exact)
      c8 = fp8e4m3 +-1.0 from the label bit
    Device per tile [128, RB, 1000]:
      u16 = Exp(z8)            (ACT, bf16 out)
      a8  = Ln(u16 + 1)        (ACT, fp8 out)  == softplus(z)
      w8  = c8 * a8            (DVE, fp8)
      per-class sums via fp8 DoubleRow ones-matmuls into f32 PSUM:
        Sa = sum a, Sw = sum w, Sc = sum c   (2 row-blocks per PE pass)
    Host combine: B=(Sa-Sw)/2, C=(Sa+Sw)/2, n_pos=(N - Sc)/2, then the
    per-class division + valid-class mean (f64).

    fp8 keeps total HBM traffic at 2 bytes/element (8.2 MB/core ~ 23 us
    at 358 GB/s); numpy sim of the full quantized pipeline shows final
    rel err ~1.2e-3 vs the f32 reference (tolerance 2e-2).
"""
import os
import numpy as np
import ml_dtypes

import concourse.bacc as bacc
import concourse.tile as tile
from concourse import mybir
from concourse.bass_utils import run_bass_kernel_spmd

N = 32768
C = 1000
NCORES = 8
NSHARD = N // NCORES          # 4096 rows per core
P = 128                       # partitions
NBLK = NSHARD // P            # 32 row-blocks of 128 rows
HALF = C // 2                 # 500-col matmul halves (one PSUM bank each)


def _schedule():
    """Row-blocks per iteration. Tapered: small first iter so the first
    Exp starts as soon as a small DMA lands, small last iter so the
    post-ACT tail (w-mul + matmuls + copies) is short; big middle iters
    amortize per-instruction ACT overhead (~352 cycles + sems each)."""
    env = os.environ.get("KERNEL_SCHED")
    if env:
        sched = [int(x) for x in env.split(",")]
    elif os.environ.get("KERNEL_TAPER", "1") == "1":
        # Three RB-2 iters at the end: each small w-mul is absorbed into
        # the next iteration's ACT window instead of queueing the final
        # w-mul behind a 4-block one still draining on Vector.
        sched = [2] + [4] * ((NBLK - 8) // 4) + [2, 2, 2]
    else:
        sched = [4] * (NBLK // 4)
    assert sum(sched) == NBLK
    return sched

_nc_cache = None
LAST_RESULTS = None           # BassKernelResults of the most recent run (for test harness)


def _patch_act_tables():
    """Make the act-table-load inserter pick the combined exp+ln set.

    The inserter greedily takes the first act_func_set containing each
    activation's function, which lands Exp in set 0 and Ln in set 5 and
    emits a table reload before every single activation (~1.7us each).
    Stripping Exp/Ln from every set except natural_log_exp_and_others
    (positions preserved, so act_func_set_id stays consistent with
    act_info.json) forces both onto one set -> a single hoisted load.
    """
    from concourse import hw_specs
    orig = hw_specs.get_activation_tables
    target = {mybir.ActivationFunctionType.Exp, mybir.ActivationFunctionType.Ln}

    def patched(arch):
        tabs = orig(arch)
        out = {}
        for name, s in tabs.items():
            if name == "natural_log_exp_and_others":
                out[name] = s
            else:
                out[name] = s - target
        return out

    prev = bacc.get_activation_tables
    bacc.get_activation_tables = patched
    return prev


def _build():
    fp8 = mybir.dt.float8e4
    bf16 = mybir.dt.bfloat16
    u_dt = {"bf16": bf16, "fp16": mybir.dt.float16,
            "f32": mybir.dt.float32}[os.environ.get("KERNEL_U_DT", "bf16")]
    a_dt = {"fp8": fp8, "bf16": bf16}[os.environ.get("KERNEL_A_DT", "fp8")]
    use_dr = os.environ.get("KERNEL_NO_DR") != "1" and a_dt == fp8

    nc = bacc.Bacc("TRN2", target_bir_lowering=False, debug=False, num_devices=NCORES)
    z_d = nc.dram_tensor("z", [NSHARD, C], fp8, kind="ExternalInput")
    c_d = nc.dram_tensor("c", [NSHARD, C], fp8, kind="ExternalInput")
    sums = nc.dram_tensor("sums", [1, 3 * C], mybir.dt.float32, kind="ExternalOutput")

    # [P, NBLK, C] view: row r = b*P + par
    zv = z_d.ap().rearrange("(b p) f -> p b f", p=P)
    cv = c_d.ap().rearrange("(b p) f -> p b f", p=P)
    sched = _schedule()
    hoist_ldw = use_dr and os.environ.get("KERNEL_HOIST_LDW", "1") == "1"

    bufs = int(os.environ.get("KERNEL_BUFS", "4"))
    # DoubleRow fp8 requires the k-tile-pair stride to be a multiple of 16
    # bytes on both the LDWEIGHTS and matmul APs -> pad the per-block class
    # stride to 1024 (fp8 = 1 B/elem).
    CP = 1024 if use_dr else C
    RBMAX = max(sched)
    FW = RBMAX * CP
    w_mode = os.environ.get("KERNEL_W_ENGINE", "vector")
    DR = mybir.MatmulPerfMode.DoubleRow

    def _no_ldw(mm):
        if hoist_ldw:
            mm.ins.ldweights = False

    with tile.TileContext(nc) as tc:
        with (
            tc.tile_pool(name="work", bufs=bufs) as work,
            tc.tile_pool(name="singles", bufs=1) as singles,
            tc.tile_pool(name="psum", bufs=1, space="PSUM") as psum,
        ):
            ones = singles.tile([P, 1], mybir.dt.float32)
            nc.vector.memset(ones, 1.0)
            ones8 = singles.tile([P, 32], fp8)
            nc.vector.memset(ones8, 1.0)
            # [128, 2, 1] weights view with a 16 B k-tile-pair stride
            o3 = ones8.rearrange("p (b f) -> p b f", f=16)[:, :, 0:1]

            # Warm the exp+ln activation table off the critical path (the
            # table load is hoisted before this first tiny activation and
            # runs in the shadow of the start barrier + first DMA).
            warm = singles.tile([1, 8], mybir.dt.float32)
            nc.vector.memset(warm, 1.0)
            nc.scalar.activation(warm, warm, mybir.ActivationFunctionType.Exp)

            if hoist_ldw:
                # All reductions share the same ones-vector weights: load
                # the PE array once, then every matmul skips its LDWEIGHTS.
                nc.tensor.ldweights(o3, perf_mode=DR)

            ps = {q: [psum.tile([1, HALF], mybir.dt.float32, name=f"ps_{q}{h}")
                      for h in range(2)] for q in ("a", "w", "c")}

            s = 0
            for i, k in enumerate(sched):
                zt = work.tile([P, k * CP], fp8, tag="zt")
                ct = work.tile([P, k * CP], fp8, tag="ct")
                z3 = zt.rearrange("p (b f) -> p b f", b=k)
                c3 = ct.rearrange("p (b f) -> p b f", b=k)
                nc.sync.dma_start(out=z3[:, :, 0:C], in_=zv[:, s:s + k])
                nc.sync.dma_start(out=c3[:, :, 0:C], in_=cv[:, s:s + k])

                # u = exp(z); a = ln(u + 1) = softplus(z)
                ut = work.tile([P, k * CP], u_dt, tag="ut")
                u3 = ut.rearrange("p (b f) -> p b f", b=k)
                nc.scalar.activation(
                    u3[:, :, 0:C], z3[:, :, 0:C], mybir.ActivationFunctionType.Exp
                )
                at = work.tile([P, k * CP], a_dt, tag="at")
                a3 = at.rearrange("p (b f) -> p b f", b=k)
                nc.scalar.activation(
                    a3[:, :, 0:C], u3[:, :, 0:C],
                    mybir.ActivationFunctionType.Ln, bias=ones, scale=1.0,
                )
                # w = c * a
                wt = work.tile([P, k * CP], a_dt, tag="wt")
                w3 = wt.rearrange("p (b f) -> p b f", b=k)
                w_eng = {"vector": nc.vector, "pool": nc.gpsimd}[w_mode]
                w_eng.tensor_mul(w3[:, :, 0:C], c3[:, :, 0:C], a3[:, :, 0:C])

                first_it = i == 0
                last_it = i == len(sched) - 1
                if use_dr and k % 2 == 0:
                    # DoubleRow: moving [128, 2, 500] sums 2 row-blocks/pass
                    for bp in range(k // 2):
                        bs = slice(2 * bp, 2 * bp + 2)
                        st = first_it and bp == 0
                        sp = last_it and bp == k // 2 - 1
                        for h in range(2):
                            cs = slice(h * HALF, (h + 1) * HALF)
                            for q, t3 in (("a", a3), ("w", w3), ("c", c3)):
                                mm = nc.tensor.matmul(
                                    ps[q][h], o3, t3[:, bs, cs], start=st, stop=sp,
                                    perf_mode=DR,
                                )
                                _no_ldw(mm)
                else:
                    for b in range(k):
                        st = first_it and b == 0
                        sp = last_it and b == k - 1
                        for h in range(2):
                            cs = slice(h * HALF, (h + 1) * HALF)
                            for q, t3 in (("a", a3), ("w", w3), ("c", c3)):
                                nc.tensor.matmul(
                                    ps[q][h], ones8[:, 0:1], t3[:, b, cs],
                                    start=st, stop=sp,
                                )
                s += k

            # PSUM->SBUF copies on the (post-Ln idle) Scalar queue, in a,c,w
            # order: only the w sums are gated by the final matmul, so they
            # go last and the a/c copies clear the queue early.
            so = singles.tile([1, 3 * C], mybir.dt.float32)
            for q in ("a", "c", "w"):
                base = {"a": 0, "w": C, "c": 2 * C}[q]
                for h in range(2):
                    dst = slice(base + h * HALF, base + (h + 1) * HALF)
                    nc.scalar.copy(so[:, dst], ps[q][h])
            nc.sync.dma_start(out=sums.ap(), in_=so)

    prev_tables = _patch_act_tables()
    try:
        nc.compile()
    finally:
        bacc.get_activation_tables = prev_tables
    return nc


def _encode_inputs(pred_y, true_y):
    """Byte-level re-encodings: z8 = fp8(c*p) via sign XOR, c8 = +-1 fp8."""
    fp8 = ml_dtypes.float8_e4m3
    tb = true_y.astype(np.uint8)
    p8 = pred_y.astype(fp8)
    z8 = (p8.view(np.uint8) ^ (tb << 7)).view(fp8)
    c8 = (0x38 | (tb << 7)).view(fp8)  # +1.0 = 0x38, -1.0 = 0xB8
    return z8, c8


def kernel(pred_y, true_y):
    global _nc_cache, LAST_RESULTS
    pred_y = np.asarray(pred_y, dtype=np.float32)
    true_y = np.asarray(true_y, dtype=np.int32)
    assert pred_y.shape == (N, C) and true_y.shape == (N, C)

    if _nc_cache is None:
        _nc_cache = _build()
    nc = _nc_cache

    z8, c8 = _encode_inputs(pred_y, true_y)
    in_maps = [
        {
            "z": np.ascontiguousarray(z8[k * NSHARD:(k + 1) * NSHARD]),
            "c": np.ascontiguousarray(c8[k * NSHARD:(k + 1) * NSHARD]),
        }
        for k in range(NCORES)
    ]

    trace = os.environ.get("KERNEL_TRACE") == "1"
    if trace:
        try:
            from antenv.axon_hooks import get_axon_ntff_profile_hook
            trace = get_axon_ntff_profile_hook() is not None
        except ImportError:
            trace = False
    res = run_bass_kernel_spmd(
        nc, in_maps, core_ids=list(range(NCORES)), trace=trace
    )
    LAST_RESULTS = res

    S = np.stack([r["sums"][0] for r in res.results]).astype(np.float64)  # [8, 3C]
    tot = S.sum(axis=0)
    Sa, Sw, Sc = tot[0:C], tot[C:2 * C], tot[2 * C:3 * C]
    B = (Sa - Sw) / 2.0
    Cn = (Sa + Sw) / 2.0
    n_pos = (N - Sc) / 2.0
    n_neg = (N + Sc) / 2.0
    valid = (n_pos > 0) & (n_neg > 0)
    loss_c = B / np.maximum(n_pos, 1.0) + Cn / np.maximum(n_neg, 1.0)
    n_valid = max(float(valid.sum()), 1.0)
    out = np.where(valid, loss_c, 0.0).sum() / n_valid
    return np.float32(out)



# revision 2
# speedup vs baseline: 1.1840x; 1.1840x over previous
"""TRN2 Bass kernel for nn_NaiveReweightedLoss (reweighted per-class BCE-style loss).

Reference semantics (N=32768 samples, C=1000 classes, t in {0,1}):
    B_c = sum_i t * softplus(-p),  C_c = sum_i (1-t) * softplus(p)
    n_pos_c = sum_i t, n_neg_c = N - n_pos_c
    valid = (n_pos>0)&(n_neg>0)
    loss = mean over valid classes of B/max(n_pos,1) + C/max(n_neg,1)

Device algorithm (data-parallel over rows, 8 cores x 4096 rows):
  Even/odd split of softplus kills one of the two ACT passes the exact
  exp+ln route needs:  softplus(z) = z/2 + E(m),  m = -|z|,
  E(m) = ln(2 cosh(m/2)) is EVEN, approximated by one tabled function:
      E(m) ~= al*silu(s*m + b) + c0        (|err| <= 0.019 on |z|<=6.5,
                                            half-normal-weighted bias ~ 0)
  Host re-encodes (byte-level only): z8 = fp8(c*p) via sign XOR, c8 = +-1.
  Device per tile [128, RB, 1000]:
      m8  = z8 | 0x80                      (DVE int32 bitwise, 4 B/elem packed)
      F8  = silu(s*m8 + b)                 (ACT, the single activation pass)
      p8  = (c8 & 0x80) ^ z8               (DVE scalar_tensor_tensor int32)
      cF8 = (c8 & 0x80) ^ F8               (DVE scalar_tensor_tensor int32)
      per-class sums of {z, p, F, cF, c} via fp8 matmuls with ONE-HOT
      [128,32] weights: quantity q lands in PSUM row 32*j + q where j is the
      column-group of the 4-way tile_position col tiling (4 row-blocks in
      flight concurrently on the PE array); 2 PSUM banks total.
  Host combine: Sa = z/2-sum + al*F-sum + c0*N, Sw = p/2-sum + al*cF-sum
  + c0*Sc, then the exact per-class division + valid-class mean (f64).

  numpy sim of the full quantized pipeline vs the f32 reference shows final
  rel err ~6e-4 (tolerance 2e-2). HBM traffic 2 B/elem (~23 us/core floor).
"""
import os
import numpy as np
import ml_dtypes

import concourse.bacc as bacc
import concourse.tile as tile
from concourse import mybir
from concourse.bass_utils import run_bass_kernel_spmd

N = 32768
C = 1000
NCORES = 8
NSHARD = N // NCORES          # 4096 rows per core
P = 128                       # partitions
NBLK = NSHARD // P            # 32 row-blocks of 128 rows
HALF = C // 2                 # 500-col matmul halves (one PSUM bank each)
NT = 4                        # col-tile groups (concurrent matmuls)

# silu fit of E(m) = ln(2cosh(m/2)) on m in [-6.5, 0], half-normal weight
AL = 0.648334
FS = -0.699517
FB = -0.743431
C0 = 0.833047

SIGN32 = -2139062144          # 0x80808080 as signed int32


def _schedule():
    env = os.environ.get("KERNEL_SCHED")
    if env:
        sched = [int(x) for x in env.split(",")]
    else:
        # small first iter so the first ACT starts as soon as a small DMA
        # lands; big middle iters amortize per-instruction ACT overhead.
        sched = [2, 2] + [4] * ((NBLK - 8) // 4) + [2, 2]
    assert sum(sched) == NBLK
    return sched


_nc_cache = None
LAST_RESULTS = None           # BassKernelResults of the most recent run (for test harness)


def _build():
    fp8 = mybir.dt.float8e4
    i32 = mybir.dt.int32
    f32 = mybir.dt.float32
    Silu = mybir.ActivationFunctionType.Silu
    XOR = mybir.AluOpType.bitwise_xor
    AND = mybir.AluOpType.bitwise_and
    OR = mybir.AluOpType.bitwise_or

    p_eng = os.environ.get("KERNEL_P_ENGINE", "vector")
    m_eng = os.environ.get("KERNEL_M_ENGINE", "vector")
    bufs = int(os.environ.get("KERNEL_BUFS", "3"))

    nc = bacc.Bacc("TRN2", target_bir_lowering=False, debug=False, num_devices=NCORES)
    z_d = nc.dram_tensor("z", [NSHARD, C], fp8, kind="ExternalInput")
    c_d = nc.dram_tensor("c", [NSHARD, C], fp8, kind="ExternalInput")
    sums = nc.dram_tensor("sums", [P, 2 * 512], f32, kind="ExternalOutput")

    zv = z_d.ap().rearrange("(b p) f -> p b f", p=P)
    cv = c_d.ap().rearrange("(b p) f -> p b f", p=P)
    sched = _schedule()

    with tile.TileContext(nc) as tc:
        with (
            tc.tile_pool(name="work", bufs=bufs) as work,
            tc.tile_pool(name="singles", bufs=1) as singles,
            tc.tile_pool(name="psum", bufs=1, space="PSUM") as psum,
        ):
            bias = singles.tile([P, 1], f32)
            nc.vector.memset(bias, FB)
            msk = singles.tile([P, 1], i32)
            nc.vector.memset(msk, SIGN32)
            # one-hot [128, 32] fp8 weights, one per summed quantity
            whot = singles.tile([P, 5 * 32], fp8)
            nc.vector.memset(whot, 0.0)
            w3 = whot.rearrange("p (q f) -> p q f", q=5)
            for q in range(5):
                nc.vector.memset(w3[:, q, q:q + 1], 1.0)

            # Warm the silu table off the critical path (hoisted table load
            # runs in the shadow of the start barrier + first DMA).
            warm = singles.tile([1, 8], f32)
            nc.vector.memset(warm, 1.0)
            nc.scalar.activation(warm, warm, Silu)

            ps = [psum.tile([P, 512], f32, name=f"ps{h}") for h in range(2)]

            # start/stop bookkeeping per (tile j, half h) accumulation region
            started = [[False] * 2 for _ in range(NT)]
            n_mm = [[0] * 2 for _ in range(NT)]
            for b in range(NBLK):
                n_mm[b % NT][0] += 5
                n_mm[b % NT][1] += 5
            seen = [[0] * 2 for _ in range(NT)]

            s = 0
            for i, k in enumerate(sched):
                zt = work.tile([P, k * C], fp8, tag="zt")
                ct = work.tile([P, k * C], fp8, tag="ct")
                z3 = zt.rearrange("p (b f) -> p b f", b=k)
                c3 = ct.rearrange("p (b f) -> p b f", b=k)
                nc.sync.dma_start(out=z3, in_=zv[:, s:s + k])
                nc.sync.dma_start(out=c3, in_=cv[:, s:s + k])

                mt = work.tile([P, k * C], fp8, tag="mt")
                ft = work.tile([P, k * C], fp8, tag="ft")
                pt = work.tile([P, k * C], fp8, tag="pt")
                cft = work.tile([P, k * C], fp8, tag="cft")

                me = {"vector": nc.vector, "pool": nc.gpsimd}[m_eng]
                me.tensor_scalar(mt.bitcast(i32), zt.bitcast(i32), msk, None, OR)
                nc.scalar.activation(ft, mt, Silu, bias=bias, scale=FS)
                pe_ = {"vector": nc.vector, "pool": nc.gpsimd}[p_eng]
                pe_.scalar_tensor_tensor(
                    pt.bitcast(i32), ct.bitcast(i32), msk, zt.bitcast(i32), AND, XOR
                )
                nc.vector.scalar_tensor_tensor(
                    cft.bitcast(i32), ct.bitcast(i32), msk, ft.bitcast(i32), AND, XOR
                )

                f3 = ft.rearrange("p (b f) -> p b f", b=k)
                p3 = pt.rearrange("p (b f) -> p b f", b=k)
                cf3 = cft.rearrange("p (b f) -> p b f", b=k)
                # z, p, c do not wait on ACT; F, cF go last
                quants = ((0, z3), (1, p3), (4, c3), (2, f3), (3, cf3))
                for bl in range(k):
                    j = (s + bl) % NT
                    for q, t3 in quants:
                        for h in range(2):
                            cs = slice(h * HALF, (h + 1) * HALF)
                            st = not started[j][h]
                            started[j][h] = True
                            seen[j][h] += 1
                            sp = seen[j][h] == n_mm[j][h]
                            nc.tensor.matmul(
                                ps[h][32 * j:32 * j + 32, 0:HALF],
                                w3[:, q, :],
                                t3[:, bl, cs],
                                start=st, stop=sp,
                                tile_position=(0, 32 * j),
                            )
                s += k

            so = singles.tile([P, 2 * 512], f32)
            for h in range(2):
                nc.scalar.copy(so[:, h * 512:(h + 1) * 512], ps[h])
            nc.sync.dma_start(out=sums.ap(), in_=so)

    nc.compile()
    return nc


def _encode_inputs(pred_y, true_y):
    """Byte-level re-encodings: z8 = fp8(c*p) via sign XOR, c8 = +-1 fp8."""
    fp8 = ml_dtypes.float8_e4m3
    tb = true_y.astype(np.uint8)
    p8 = pred_y.astype(fp8)
    z8 = (p8.view(np.uint8) ^ (tb << 7)).view(fp8)
    c8 = (0x38 | (tb << 7)).view(fp8)  # +1.0 = 0x38, -1.0 = 0xB8
    return z8, c8


def kernel(pred_y, true_y):
    global _nc_cache, LAST_RESULTS
    pred_y = np.asarray(pred_y, dtype=np.float32)
    true_y = np.asarray(true_y, dtype=np.int32)
    assert pred_y.shape == (N, C) and true_y.shape == (N, C)

    if _nc_cache is None:
        _nc_cache = _build()
    nc = _nc_cache

    z8, c8 = _encode_inputs(pred_y, true_y)
    in_maps = [
        {
            "z": np.ascontiguousarray(z8[k * NSHARD:(k + 1) * NSHARD]),
            "c": np.ascontiguousarray(c8[k * NSHARD:(k + 1) * NSHARD]),
        }
        for k in range(NCORES)
    ]

    trace = os.environ.get("KERNEL_TRACE") == "1"
    if trace:
        try:
            from antenv.axon_hooks import get_axon_ntff_profile_hook
            trace = get_axon_ntff_profile_hook() is not None
        except ImportError:
            trace = False
    res = run_bass_kernel_spmd(
        nc, in_maps, core_ids=list(range(NCORES)), trace=trace
    )
    LAST_RESULTS = res

    S = np.stack([r["sums"] for r in res.results]).astype(np.float64)  # [8, 128, 1024]
    tot = S.sum(axis=0)
    V = np.zeros((5, C))
    for q in range(5):
        for h in range(2):
            acc = np.zeros(HALF)
            for j in range(NT):
                acc += tot[32 * j + q, h * 512:h * 512 + HALF]
            V[q, h * HALF:(h + 1) * HALF] = acc
    Sz, Sp, SF, ScF, Sc = V

    Sa = 0.5 * Sz + AL * SF + C0 * N
    Sw = 0.5 * Sp + AL * ScF + C0 * Sc
    B = (Sa - Sw) / 2.0
    Cn = (Sa + Sw) / 2.0
    n_pos = (N - Sc) / 2.0
    n_neg = (N + Sc) / 2.0
    valid = (n_pos > 0) & (n_neg > 0)
    loss_c = B / np.maximum(n_pos, 1.0) + Cn / np.maximum(n_neg, 1.0)
    n_valid = max(float(valid.sum()), 1.0)
    out = np.where(valid, loss_c, 0.0).sum() / n_valid
    return np.float32(out)


# revision 4
# speedup vs baseline: 1.4464x; 1.2216x over previous
"""TRN2 Bass kernel for nn_NaiveReweightedLoss (reweighted per-class BCE-style loss).

Reference semantics (N=32768 samples, C=1000 classes, t in {0,1}):
    B_c = sum_i t * softplus(-p),  C_c = sum_i (1-t) * softplus(p)
    n_pos_c = sum_i t, n_neg_c = N - n_pos_c
    valid = (n_pos>0)&(n_neg>0)
    loss = mean over valid classes of B/max(n_pos,1) + C/max(n_neg,1)

Device algorithm (data-parallel over rows, 8 cores x 4096 rows):
  Even/odd split of softplus kills one of the two ACT passes the exact
  exp+ln route needs:  softplus(z) = z/2 + E(m),  m = -|z|,
  E(m) = ln(2 cosh(m/2)) is EVEN, approximated by one tabled function:
      E(m) ~= al*silu(s*m + b) + c0        (|err| <= 0.019 on |z|<=6.5,
                                            half-normal-weighted bias ~ 0)
  Host re-encodes (byte-level only): z8 = fp8(c*p) via sign XOR, c8 = +-1.
  Device per tile [128, RB, 1000]:
      m8  = z8 | 0x80                      (DVE int32 bitwise, 4 B/elem packed)
      F8  = silu(s*m8 + b)                 (ACT, the single activation pass)
      p8  = (c8 & 0x80) ^ z8               (DVE scalar_tensor_tensor int32)
      cF8 = (c8 & 0x80) ^ F8               (DVE scalar_tensor_tensor int32)
      per-class sums of {z, p, F, cF, c} via fp8 matmuls with ONE-HOT
      [128,32] weights: quantity q lands in PSUM row 32*j + q where j is the
      column-group of the 4-way tile_position col tiling (4 row-blocks in
      flight concurrently on the PE array); 2 PSUM banks total.
  Host combine: Sa = z/2-sum + al*F-sum + c0*N, Sw = p/2-sum + al*cF-sum
  + c0*Sc, then the exact per-class division + valid-class mean (f64).

  numpy sim of the full quantized pipeline vs the f32 reference shows final
  rel err ~6e-4 (tolerance 2e-2). HBM traffic 2 B/elem (~23 us/core floor).
"""
import os
import numpy as np
import ml_dtypes

import concourse.bacc as bacc
import concourse.tile as tile
from concourse import mybir
from concourse.bass_utils import run_bass_kernel_spmd

N = 32768
C = 1000
NCORES = 8
NSHARD = N // NCORES          # 4096 rows per core
P = 128                       # partitions
NBLK = NSHARD // P            # 32 row-blocks of 128 rows
HALF = C // 2                 # 500-col matmul halves (one PSUM bank each)
NT = 4                        # col-tile groups (concurrent matmuls)

# silu fit of E(m) = ln(2cosh(m/2)) on m in [-6.5, 0], half-normal weight
AL = 0.648334
FS = -0.699517
FB = -0.743431
C0 = 0.833047

SIGN32 = -2139062144          # 0x80808080 as signed int32


def _schedule():
    env = os.environ.get("KERNEL_SCHED")
    if env:
        sched = [int(x) for x in env.split(",")]
    else:
        # small first iter so the first ACT starts as soon as a small DMA
        # lands; big middle iters amortize per-instruction ACT overhead.
        sched = [2, 2, 4] + [8] * 2 + [4, 2, 2]
    assert sum(sched) == NBLK
    return sched


_nc_cache = None
LAST_RESULTS = None           # BassKernelResults of the most recent run (for test harness)


def _build():
    fp8 = mybir.dt.float8e4
    i32 = mybir.dt.int32
    f32 = mybir.dt.float32
    Silu = mybir.ActivationFunctionType.Silu
    XOR = mybir.AluOpType.bitwise_xor
    AND = mybir.AluOpType.bitwise_and
    OR = mybir.AluOpType.bitwise_or

    p_eng = os.environ.get("KERNEL_P_ENGINE", "vector")
    m_eng = os.environ.get("KERNEL_M_ENGINE", "vector")
    bufs = int(os.environ.get("KERNEL_BUFS", "3"))

    nc = bacc.Bacc("TRN2", target_bir_lowering=False, debug=False, num_devices=NCORES)
    z_d = nc.dram_tensor("z", [NSHARD, C], fp8, kind="ExternalInput")
    c_d = nc.dram_tensor("c", [NSHARD, C], fp8, kind="ExternalInput")
    sums = nc.dram_tensor("sums", [P, 2 * 512], f32, kind="ExternalOutput")

    zv = z_d.ap().rearrange("(b p) f -> p b f", p=P)
    cv = c_d.ap().rearrange("(b p) f -> p b f", p=P)
    sched = _schedule()

    with tile.TileContext(nc) as tc:
        with (
            tc.tile_pool(name="work", bufs=bufs) as work,
            tc.tile_pool(name="singles", bufs=1) as singles,
            tc.tile_pool(name="psum", bufs=1, space="PSUM") as psum,
        ):
            bias = singles.tile([P, 1], f32)
            nc.vector.memset(bias, FB)
            msk = singles.tile([P, 1], i32)
            nc.vector.memset(msk, SIGN32)
            # one-hot [128, 32] fp8 weights, one per summed quantity
            whot = singles.tile([P, 5 * 32], fp8)
            nc.vector.memset(whot, 0.0)
            w3 = whot.rearrange("p (q f) -> p q f", q=5)
            for q in range(5):
                nc.vector.memset(w3[:, q, q:q + 1], 1.0)

            # Warm the silu table off the critical path (hoisted table load
            # runs in the shadow of the start barrier + first DMA).
            warm = singles.tile([1, 8], f32)
            nc.vector.memset(warm, 1.0)
            nc.scalar.activation(warm, warm, Silu)

            ps = [psum.tile([P, 512], f32, name=f"ps{h}") for h in range(2)]

            # start/stop bookkeeping per (tile j, half h) accumulation region
            started = [[False] * 2 for _ in range(NT)]
            n_mm = [[0] * 2 for _ in range(NT)]
            for b in range(NBLK):
                n_mm[b % NT][0] += 5
                n_mm[b % NT][1] += 5
            seen = [[0] * 2 for _ in range(NT)]

            s = 0
            for i, k in enumerate(sched):
                zt = work.tile([P, k * C], fp8, tag="zt")
                ct = work.tile([P, k * C], fp8, tag="ct")
                z3 = zt.rearrange("p (b f) -> p b f", b=k)
                c3 = ct.rearrange("p (b f) -> p b f", b=k)
                nc.sync.dma_start(out=z3, in_=zv[:, s:s + k])
                nc.sync.dma_start(out=c3, in_=cv[:, s:s + k])

                mt = work.tile([P, k * C], fp8, tag="mt")
                ft = work.tile([P, k * C], fp8, tag="ft")
                pt = work.tile([P, k * C], fp8, tag="pt")
                cft = work.tile([P, k * C], fp8, tag="cft")

                me = {"vector": nc.vector, "pool": nc.gpsimd}[m_eng]
                me.tensor_scalar(mt.bitcast(i32), zt.bitcast(i32), msk, None, OR)
                nc.scalar.activation(ft, mt, Silu, bias=bias, scale=FS)
                pe_ = {"vector": nc.vector, "pool": nc.gpsimd}[p_eng]
                pe_.scalar_tensor_tensor(
                    pt.bitcast(i32), ct.bitcast(i32), msk, zt.bitcast(i32), AND, XOR
                )
                nc.vector.scalar_tensor_tensor(
                    cft.bitcast(i32), ct.bitcast(i32), msk, ft.bitcast(i32), AND, XOR
                )

                f3 = ft.rearrange("p (b f) -> p b f", b=k)
                p3 = pt.rearrange("p (b f) -> p b f", b=k)
                cf3 = cft.rearrange("p (b f) -> p b f", b=k)
                # z, p, c do not wait on ACT; F, cF go last. Block-inner so
                # consecutive matmuls hit different col-groups and overlap
                # on the PE array (4-way tile concurrency).
                quants = ((0, z3), (1, p3), (4, c3), (2, f3), (3, cf3))
                for q, t3 in quants:
                    for h in range(2):
                        cs = slice(h * HALF, (h + 1) * HALF)
                        for bl in range(k):
                            j = (s + bl) % NT
                            st = not started[j][h]
                            started[j][h] = True
                            seen[j][h] += 1
                            sp = seen[j][h] == n_mm[j][h]
                            nc.tensor.matmul(
                                ps[h][32 * j:32 * j + 32, 0:HALF],
                                w3[:, q, :],
                                t3[:, bl, cs],
                                start=st, stop=sp,
                                tile_position=(0, 32 * j),
                            )
                s += k

            so = singles.tile([P, 2 * 512], f32)
            for h in range(2):
                nc.scalar.copy(so[:, h * 512:(h + 1) * 512], ps[h])
            nc.sync.dma_start(out=sums.ap(), in_=so)

    nc.compile()
    return nc


def _encode_inputs(pred_y, true_y):
    """Byte-level re-encodings: z8 = fp8(c*p) via sign XOR, c8 = +-1 fp8."""
    fp8 = ml_dtypes.float8_e4m3
    tb = true_y.astype(np.uint8)
    p8 = pred_y.astype(fp8)
    z8 = (p8.view(np.uint8) ^ (tb << 7)).view(fp8)
    c8 = (0x38 | (tb << 7)).view(fp8)  # +1.0 = 0x38, -1.0 = 0xB8
    return z8, c8


def kernel(pred_y, true_y):
    global _nc_cache, LAST_RESULTS
    pred_y = np.asarray(pred_y, dtype=np.float32)
    true_y = np.asarray(true_y, dtype=np.int32)
    assert pred_y.shape == (N, C) and true_y.shape == (N, C)

    if _nc_cache is None:
        _nc_cache = _build()
    nc = _nc_cache

    z8, c8 = _encode_inputs(pred_y, true_y)
    in_maps = [
        {
            "z": np.ascontiguousarray(z8[k * NSHARD:(k + 1) * NSHARD]),
            "c": np.ascontiguousarray(c8[k * NSHARD:(k + 1) * NSHARD]),
        }
        for k in range(NCORES)
    ]

    trace = os.environ.get("KERNEL_TRACE") == "1"
    if trace:
        try:
            from antenv.axon_hooks import get_axon_ntff_profile_hook
            trace = get_axon_ntff_profile_hook() is not None
        except ImportError:
            trace = False
    res = run_bass_kernel_spmd(
        nc, in_maps, core_ids=list(range(NCORES)), trace=trace
    )
    LAST_RESULTS = res

    S = np.stack([r["sums"] for r in res.results]).astype(np.float64)  # [8, 128, 1024]
    tot = S.sum(axis=0)
    V = np.zeros((5, C))
    for q in range(5):
        for h in range(2):
            acc = np.zeros(HALF)
            for j in range(NT):
                acc += tot[32 * j + q, h * 512:h * 512 + HALF]
            V[q, h * HALF:(h + 1) * HALF] = acc
    Sz, Sp, SF, ScF, Sc = V

    Sa = 0.5 * Sz + AL * SF + C0 * N
    Sw = 0.5 * Sp + AL * ScF + C0 * Sc
    B = (Sa - Sw) / 2.0
    Cn = (Sa + Sw) / 2.0
    n_pos = (N - Sc) / 2.0
    n_neg = (N + Sc) / 2.0
    valid = (n_pos > 0) & (n_neg > 0)
    loss_c = B / np.maximum(n_pos, 1.0) + Cn / np.maximum(n_neg, 1.0)
    n_valid = max(float(valid.sum()), 1.0)
    out = np.where(valid, loss_c, 0.0).sum() / n_valid
    return np.float32(out)
